# revision 27
# baseline (speedup 1.0000x reference)
"""Dense MoE layer (8 experts, all-expert weighted combine) on 8 TRN2 NeuronCores.

Strategy: data-parallel over the token dim. Each core gets a 1024-token shard
(pre-transposed + bf16-cast on host), the full stacked expert weights (bf16),
and computes gate softmax + all 8 expert matmuls + gate-weighted combine
locally. No collectives; host concatenates the 8 output shards.

Per-core device schedule (software-pipelined over token tiles t):
  - y(e,t) = x_t @ We[e]: 16 matmuls (8 K-chunks x 2 N=512 halves) accumulate
    in PSUM f32; combine is one fused DVE op: out = psum*g[:,e] + out.
  - gate logits are computed TRANSPOSED (lhsT = Wg chunk, 8-column weight
    loads are ~free) into [8,128] psum, bias-added as a per-partition scalar,
    then DVE 32x32-block-transposed back to [128,8] for the free-dim softmax.
    exp uses accum_out to produce the softmax denominator for free.
  - the bias term g@be is a K=8 matmul against the transposed gate, into two
    [128,512] psum half-tiles (bufs=2) so its WAR never stalls the PE.
  - pipeline: block t runs y(0,t) while softmax(t) resolves and bias(t-1)
    lands. Keeps PE gap-free.
  - HAM warmup/fill matmuls (N=128, cheap) keep the PE clock gate at 2.4 GHz
    through the HBM-bound ramp-in while the first inputs stream.
  - the last tile of the last expert is split into 4 N=256 quarter-chains
    with stall-free psum slots so combines + writeback overlap the matmuls;
    only the final quarter remains on the end-of-kernel critical path.

DMA issue costs ~0.6us per dma_start on the issuing engine. The startup is
HBM-bandwidth-bound on xA+We[0] (3MB): those are interleaved in consumption
order ACROSS sync and scalar (1.5MB each) so both queues carry critical
bytes; xB and We[1] are queued strictly after so they don't steal bandwidth
from the critical window.
"""

import os
import sys

import numpy as np

try:
    import concourse.bass as bass  # noqa: F401
except ImportError:  # harness containers stage the repo at /opt/trn_rl_repo
    sys.path.insert(0, "/opt/trn_rl_repo")

from contextlib import ExitStack

import ml_dtypes

import concourse.bass as bass
import concourse.mybir as mybir
import concourse.tile as tile
from concourse import bacc
from concourse.bass_utils import run_bass_kernel_spmd

N_CORES = 8
N_TOK = 8192
IN_F = 1024
OUT_F = 1024
E = 8
P = 128  # partitions


def build_nc(n_tok_pc: int = N_TOK // N_CORES, debug: bool = False):
    """Build the single-core SPMD Bass program (same program on all 8 cores)."""
    fp32 = mybir.dt.float32
    bf16 = mybir.dt.bfloat16

    K_CH = IN_F // P  # contraction chunks of 128
    T = n_tok_pc // P  # token tiles per core
    assert T >= 2

    nc = bacc.Bacc(
        "TRN2", target_bir_lowering=False, debug=debug, enable_asserts=False
    )

    xT = nc.declare_dram_parameter("xT", [IN_F, n_tok_pc], bf16, isOutput=False)
    We = nc.declare_dram_parameter("We", [E, IN_F, OUT_F], bf16, isOutput=False)
    be = nc.declare_dram_parameter("be", [E, OUT_F], bf16, isOutput=False)
    Wg = nc.declare_dram_parameter("Wg", [P, K_CH, E], bf16, isOutput=False)
    bgc = nc.declare_dram_parameter("bgc", [E, 1], fp32, isOutput=False)
    out = nc.declare_dram_parameter("out", [n_tok_pc, OUT_F], fp32, isOutput=True)

    with tile.TileContext(nc) as tc, ExitStack() as ctx:
        consts = ctx.enter_context(tc.tile_pool(name="consts", bufs=1))
        xpool = ctx.enter_context(tc.tile_pool(name="xpool", bufs=1))
        wepool = ctx.enter_context(tc.tile_pool(name="wepool", bufs=2))
        opool = ctx.enter_context(tc.tile_pool(name="opool", bufs=1))
        gpool = ctx.enter_context(tc.tile_pool(name="gpool", bufs=1))
        small = ctx.enter_context(tc.tile_pool(name="small", bufs=4))
        # 8 PSUM banks: 5 x yh ([128,512] f32 half-accumulators — the deep
        # rotation pushes the block-start WAR ~2.5 blocks back so it never
        # reaches the PE) + 2 x pb (bias halves, double-buffered) + 1 x lgt.
        psum_yh = ctx.enter_context(tc.tile_pool(name="psum_yh", bufs=5, space="PSUM"))
        psum_pb = ctx.enter_context(tc.tile_pool(name="psum_pb", bufs=2, space="PSUM"))
        psum_g = ctx.enter_context(tc.tile_pool(name="psum_g", bufs=1, space="PSUM"))

        # ---- HAM warmup: start PE activity as early as possible ----
        # gpsimd memset is available right after the preamble barrier
        # (vector is still busy with its register loads then), so warm
        # matmuls can begin ~1us earlier. N=128 keeps each warm matmul
        # cheap so real matmuls slot in as soon as their data lands.
        warm_sb = consts.tile([P, P], bf16)
        nc.gpsimd.memset(warm_sb, 0.25)

        def warm_fill():
            # dep-free N=128 matmul: fills DMA-chase idle so the HAM clock
            # gate never sees an idle window. Writes a fresh yh slot; its
            # only accessor is the matmul itself, so the slot frees at once.
            wps = psum_yh.tile([P, P], fp32, tag="yh")
            nc.tensor.matmul(wps, lhsT=warm_sb, rhs=warm_sb, start=True, stop=True)

        for _ in range(32):
            warm_fill()

        # ---- input DMAs ----
        # small gate constants ride gpsimd's (software) DGE — fast for small
        # transfers and keeps the sync/scalar issue streams free for the bulk
        wg_sb = consts.tile([P, K_CH, E], bf16)
        nc.gpsimd.dma_start(out=wg_sb, in_=Wg[:, :, :])
        bgc_sb = consts.tile([E, 1], fp32)
        nc.gpsimd.dma_start(out=bgc_sb, in_=bgc[:, :])
        be_sb = consts.tile([E, OUT_F], bf16)
        nc.gpsimd.dma_start(out=be_sb, in_=be[:, :])

        def fetch_we_chunk(e, c, eng=None):
            # later-expert prefetches ride SYNC: their dma_starts embed long
            # wepool-WAR waits (the slot frees only when expert e-2 finishes
            # reading it), and on scalar those waits would block the ACT
            # FIFO ahead of the per-block bias copies for microseconds.
            wc = wepool.tile([P, OUT_F], bf16, tag=f"we{c}")
            (eng or nc.sync).dma_start(
                out=wc, in_=We[e, c * P : (c + 1) * P, :]
            )
            return wc

        def fetch_we(e):
            return [fetch_we_chunk(e, c) for c in range(K_CH)]

        # The start of the kernel is HBM-bound: block 0 needs the xA
        # half-chunks AND all of We[0] (3MB critical). xA rides sync alone
        # (so the gate chain gets chunks at a fast, even pace); We[0] is
        # split 6 on scalar + the last 2 on sync behind xA, so both queues
        # carry only critical bytes until block 0's data is in. xB (needed
        # from t=T/2) queues on scalar right after its critical share, and
        # We[1] (needed at the e=1 sweep) on sync.
        nh = n_tok_pc // 2
        T_half = T // 2
        xA_sb, xB_sb = [None] * K_CH, [None] * K_CH
        we_sb = {0: [None] * K_CH}
        for c in range(K_CH):
            xa = xpool.tile([P, nh], bf16, tag=f"xa{c}")
            nc.sync.dma_start(out=xa, in_=xT[c * P : (c + 1) * P, :nh])
            xA_sb[c] = xa
            if c < 6:
                we_sb[0][c] = fetch_we_chunk(0, c, eng=nc.scalar)
        for c in range(6, K_CH):
            we_sb[0][c] = fetch_we_chunk(0, c, eng=nc.sync)
        # scalar carries NOTHING after its 6 We[0] chunks: any dma_start on
        # it would sit in the ACT FIFO ahead of the per-block bias copies.
        # xB rides sync behind We[1]; it lands ~26us, needed at t=T/2 (~28).
        we_sb[1] = [fetch_we_chunk(1, c, eng=nc.sync) for c in range(K_CH)]
        for c in range(K_CH):
            xb = xpool.tile([P, nh], bf16, tag=f"xb{c}")
            nc.sync.dma_start(out=xb, in_=xT[c * P : (c + 1) * P, nh:])
            xB_sb[c] = xb

        def xslice(c, t):
            if t < T_half:
                return xA_sb[c][:, t * P : (t + 1) * P]
            return xB_sb[c][:, (t - T_half) * P : (t - T_half + 1) * P]

        g_sb = gpool.tile([P, T, E], fp32)
        gTexp_sb = gpool.tile([E, T, P], bf16)
        # transposed exp'd gate staging in f32: 32 partitions so the DVE
        # 32x32 block transpose can address it; partitions 8..31 are zero.
        gTexp32_sb = gpool.tile([32, T, P], fp32)
        nc.gpsimd.memset(gTexp32_sb, 0.0)
        r_sb = gpool.tile([P, T], fp32)
        # out staging split into 4 tiles (t mod 4): Tile tracks deps per
        # tile, so a single out tile would falsely serialize the ACT bias
        # copies against the DVE combines of *other* token tiles.
        out4 = []
        for i in range(4):
            o_i = opool.tile([P, T // 4, OUT_F], fp32, tag=f"out{i}")
            out4.append(o_i)

        def oview(t, s=slice(None)):
            return out4[t % 4][:, t // 4, s]

        def main_mms(e, t, warm=0):
            # h-outer: each 512-wide half accumulates in its own psum tile,
            # so the h0 half finishes (and its combine starts) mid-block.
            phs = []
            for h in range(2):
                ph = psum_yh.tile([P, 512], fp32, tag="yh")
                hs = slice(h * 512, (h + 1) * 512)
                for c in range(K_CH):
                    # fills go BEFORE the matmul: the PE queue is FIFO,
                    # so a fill behind a DMA-stalled matmul can't run
                    for _ in range(warm):
                        warm_fill()
                    nc.tensor.matmul(
                        ph,
                        lhsT=xslice(c, t),
                        rhs=we_sb[e][c][:, hs],
                        start=(c == 0),
                        stop=(c == K_CH - 1),
                    )
                phs.append(ph)
            return phs

        lgt_live = {}

        def gate_mms_batch(b):
            # transposed gate logits for a whole token-half at once:
            # lhsT = Wg chunk (8-col weight load), rhs = xA/xB (N=nh moving)
            lgt = psum_g.tile([E, nh], fp32, tag="g8")
            half = xA_sb if b == 0 else xB_sb
            for c in range(K_CH):
                if b == 0:
                    # dep-free fillers BEFORE the (DMA-chasing) matmul: the
                    # PE queue is FIFO, so fills behind a stalled matmul
                    # can't keep the HAM clock gate warm
                    warm_fill()
                    warm_fill()
                    warm_fill()
                nc.tensor.matmul(
                    lgt,
                    lhsT=wg_sb[:, c, :],
                    rhs=half[c][:, :],
                    start=(c == 0),
                    stop=(c == K_CH - 1),
                )
            # += bg (per-partition scalar in transposed space)
            nc.vector.tensor_scalar_add(lgt, lgt, bgc_sb[:, :])
            lgt_live[b] = lgt

        def gate_exps(b):
            # unnormalized transposed exp (logits are O(+-3) so exp without
            # max-subtraction is safe in f32): bf16 copy feeds the bias
            # matmul, f32 copy feeds the per-tile DVE transpose + softmax.
            # Emitted a block after the batch matmuls so the ACT FIFO never
            # stalls on them ahead of the per-block bias copies.
            lgt = lgt_live.pop(b)
            bslc = slice(b * T_half, (b + 1) * T_half)
            nc.scalar.activation(
                out=gTexp_sb[:, bslc, :],
                in_=lgt,
                func=mybir.ActivationFunctionType.Exp,
            )
            nc.scalar.activation(
                out=gTexp32_sb[:E, bslc, :],
                in_=lgt,
                func=mybir.ActivationFunctionType.Exp,
            )

        def softmax(t):
            # DVE 32x32-block transpose of the exp'd gate back to [tok, e]
            # (4 blocks; input partitions 8..31 are zero so output columns
            # 8..31 are zero and unused), then an all-DVE normalize chain —
            # no scalar-engine hop on this path.
            e32 = small.tile([P, 32], fp32, tag="z32")
            for j in range(4):
                nc.vector.transpose(
                    out=e32[32 * j : 32 * (j + 1), 0:32],
                    in_=gTexp32_sb[:, t, 32 * j : 32 * (j + 1)],
                )
            ssum = small.tile([P, 1], fp32, tag="ssum")
            nc.vector.tensor_reduce(
                out=ssum, in_=e32[:, 0:E], axis=mybir.AxisListType.X,
                op=mybir.AluOpType.add,
            )
            nc.vector.reciprocal(out=r_sb[:, t : t + 1], in_=ssum)
            nc.vector.tensor_scalar_mul(g_sb[:, t, :], e32[:, 0:E], r_sb[:, t : t + 1])

        def bias_init(t):
            # unnormalized bias term exp(z) @ be, folded in normalized as
            # the INITIALIZER of out[t]: out[t] = Copy(pb * r) on the scalar
            # engine (per-partition scale); takes the bias fold off the
            # near-saturated DVE.
            for h in range(2):
                hs = slice(h * 512, (h + 1) * 512)
                pb = psum_pb.tile([P, 512], fp32, tag="pb")
                nc.tensor.matmul(
                    pb, lhsT=gTexp_sb[:, t, :], rhs=be_sb[:, hs],
                    start=True, stop=True,
                )
                nc.scalar.activation(
                    out=oview(t, hs),
                    in_=pb[:, :],
                    func=mybir.ActivationFunctionType.Copy,
                    scale=r_sb[:, t : t + 1],
                )

        def combine(e, t, phs):
            # out[t] = y(e) * g[:, t, e] + out[t]   (fused on DVE, per half;
            # for e=0 the bias term is already in out[t])
            for h, ph in enumerate(phs):
                hs = slice(h * 512, (h + 1) * 512)
                nc.vector.scalar_tensor_tensor(
                    out=oview(t, hs),
                    in0=ph[:, :],
                    scalar=g_sb[:, t, e : e + 1],
                    in1=oview(t, hs),
                    op0=mybir.AluOpType.mult,
                    op1=mybir.AluOpType.add,
                )

        combine0 = lambda t, phs: combine(0, t, phs)

        # ---- phase A: e=0 pipelined with the gate computation ----
        # block t emission order matters: the softmax chain must NOT queue
        # behind combine0 on DVE (combine0 depends on the ACT bias copy,
        # which depends on the previous recip — ordering softmax first keeps
        # the cross-engine chain's latency off the DVE FIFO head). The bias
        # matmuls+copies lead the block so their psum WAR resolves early:
        #   bias(t-1) | main(0,t) | gate mms | softmax(t) | combine0(t-1)
        py_live = {}
        gate_mms_batch(0)
        gate_exps(0)
        for t in range(T):
            if t >= 1:
                bias_init(t - 1)
            py_live[t] = main_mms(0, t, warm=(2 if t == 0 else 1 if t == 1 else 0))
            if t == max(0, T_half - 2):
                gate_mms_batch(1)
                gate_exps(1)
            softmax(t)
            if t >= 1:
                combine0(t - 1, py_live.pop(t - 1))

        # bridge: keep PE fed while softmax(T-1) resolves
        bias_init(T - 1)
        py_b = main_mms(1, 0)

        combine0(T - 1, py_live.pop(T - 1))
        combine(1, 0, py_b)

        # ---- phase B: experts 1..7 ----
        for e in range(1, E):
            if e + 1 < E:
                we_sb[e + 1] = fetch_we(e + 1)
            for t in range(1 if e == 1 else 0, T):
                if e == E - 1 and t == T - 1:
                    # final tile: 4 quarter chains (N=256) so each quarter's
                    # combine + write-back overlaps the next quarter's
                    # matmuls; the 5-deep yh rotation keeps every WAR off
                    # the PE.
                    for q in range(4):
                        qs = slice(q * 256, (q + 1) * 256)
                        pq = psum_yh.tile([P, 256], fp32, tag="yh")
                        for c in range(K_CH):
                            nc.tensor.matmul(
                                pq,
                                lhsT=xslice(c, t),
                                rhs=we_sb[e][c][:, qs],
                                start=(c == 0),
                                stop=(c == K_CH - 1),
                            )
                        nc.vector.scalar_tensor_tensor(
                            out=oview(t, qs),
                            in0=pq[:, :],
                            scalar=g_sb[:, t, e : e + 1],
                            in1=oview(t, qs),
                            op0=mybir.AluOpType.mult,
                            op1=mybir.AluOpType.add,
                        )
                        eng = nc.sync if q % 2 == 0 else nc.scalar
                        eng.dma_start(
                            out=out[t * P : (t + 1) * P, qs],
                            in_=oview(t, qs),
                        )
                else:
                    py = main_mms(e, t)
                    combine(e, t, py)
                    if e == E - 1:
                        # write back this tile right after its final combine
                        nc.sync.dma_start(
                            out=out[t * P : (t + 1) * P, :], in_=oview(t)
                        )
            del we_sb[e - 1]

    nc.compile()
    return nc


_NC_CACHE: dict = {}


def _get_nc(n_tok_pc: int):
    if n_tok_pc not in _NC_CACHE:
        _NC_CACHE[n_tok_pc] = build_nc(n_tok_pc)
    return _NC_CACHE[n_tok_pc]


def make_in_maps(x, We, be, Wg, bg):
    """Host-side sharding: token-shard + transpose x, bf16-cast everything."""
    bf16 = ml_dtypes.bfloat16
    x = np.asarray(x)
    n_tok_pc = x.shape[0] // N_CORES
    We_bf = np.asarray(We).astype(bf16)
    be_bf = np.asarray(be).astype(bf16)
    K_CH = IN_F // P
    # [1024, 8] -> [p, chunk, e]
    Wg_bf = (
        np.asarray(Wg).astype(bf16).reshape(K_CH, P, E).transpose(1, 0, 2).copy()
    )
    bg_col = np.asarray(bg).astype(np.float32).reshape(E, 1)
    xbf = x.astype(bf16)
    in_maps = []
    for cid in range(N_CORES):
        xs = xbf[cid * n_tok_pc : (cid + 1) * n_tok_pc]
        in_maps.append(
            {
                "xT": np.ascontiguousarray(xs.T),
                "We": We_bf,
                "be": be_bf,
                "Wg": Wg_bf,
                "bgc": bg_col,
            }
        )
    return in_maps, n_tok_pc


def run(x, We, be, Wg, bg, trace=False, **trace_kwargs):
    in_maps, n_tok_pc = make_in_maps(x, We, be, Wg, bg)
    nc = _get_nc(n_tok_pc)
    res = run_bass_kernel_spmd(
        nc, in_maps, core_ids=list(range(N_CORES)), trace=trace, **trace_kwargs
    )
    outs = [res.results[i]["out"] for i in range(N_CORES)]
    return np.concatenate(outs, axis=0), res


def kernel(x, We, be, Wg, bg):
    out, _ = run(x, We, be, Wg, bg, trace=False)
    return out


# revision 28
# speedup vs baseline: 1.0116x; 1.0116x over previous
"""Dense MoE layer (8 experts, all-expert weighted combine) on 8 TRN2 NeuronCores.

Strategy: data-parallel over the token dim. Each core gets a 1024-token shard
(pre-transposed + bf16-cast on host), the full stacked expert weights (bf16),
and computes gate softmax + all 8 expert matmuls + gate-weighted combine
locally. No collectives; host concatenates the 8 output shards.

Per-core device schedule (software-pipelined over token tiles t):
  - y(e,t) = x_t @ We[e]: 16 matmuls (8 K-chunks x 2 N=512 halves) accumulate
    in PSUM f32; combine is one fused DVE op: out = psum*g[:,e] + out.
  - gate logits are computed TRANSPOSED (lhsT = Wg chunk, 8-column weight
    loads are ~free) into [8,128] psum, bias-added as a per-partition scalar,
    then DVE 32x32-block-transposed back to [128,8] for the free-dim softmax.
    exp uses accum_out to produce the softmax denominator for free.
  - the bias term g@be is a K=8 matmul against the transposed gate, into two
    [128,512] psum half-tiles (bufs=2) so its WAR never stalls the PE.
  - pipeline: block t runs y(0,t) while softmax(t) resolves and bias(t-1)
    lands. Keeps PE gap-free.
  - HAM warmup/fill matmuls (N=128, cheap) keep the PE clock gate at 2.4 GHz
    through the HBM-bound ramp-in while the first inputs stream.
  - the last tile of the last expert is split into 4 N=256 quarter-chains
    with stall-free psum slots so combines + writeback overlap the matmuls;
    only the final quarter remains on the end-of-kernel critical path.

DMA issue costs ~0.6us per dma_start on the issuing engine. The startup is
HBM-bandwidth-bound on xA+We[0] (3MB): those are interleaved in consumption
order ACROSS sync and scalar (1.5MB each) so both queues carry critical
bytes; xB and We[1] are queued strictly after so they don't steal bandwidth
from the critical window.
"""

import os
import sys

import numpy as np

try:
    import concourse.bass as bass  # noqa: F401
except ImportError:  # harness containers stage the repo at /opt/trn_rl_repo
    sys.path.insert(0, "/opt/trn_rl_repo")

from contextlib import ExitStack

import ml_dtypes

import concourse.bass as bass
import concourse.mybir as mybir
import concourse.tile as tile
from concourse import bacc
from concourse.bass_utils import run_bass_kernel_spmd

N_CORES = 8
N_TOK = 8192
IN_F = 1024
OUT_F = 1024
E = 8
P = 128  # partitions


def build_nc(n_tok_pc: int = N_TOK // N_CORES, debug: bool = False):
    """Build the single-core SPMD Bass program (same program on all 8 cores)."""
    fp32 = mybir.dt.float32
    bf16 = mybir.dt.bfloat16

    K_CH = IN_F // P  # contraction chunks of 128
    T = n_tok_pc // P  # token tiles per core
    assert T >= 2

    nc = bacc.Bacc(
        "TRN2", target_bir_lowering=False, debug=debug, enable_asserts=False
    )

    xT = nc.declare_dram_parameter("xT", [IN_F, n_tok_pc], bf16, isOutput=False)
    We = nc.declare_dram_parameter("We", [E, IN_F, OUT_F], bf16, isOutput=False)
    be = nc.declare_dram_parameter("be", [E, OUT_F], bf16, isOutput=False)
    Wg = nc.declare_dram_parameter("Wg", [P, K_CH, E], bf16, isOutput=False)
    bgc = nc.declare_dram_parameter("bgc", [E, 1], fp32, isOutput=False)
    out = nc.declare_dram_parameter("out", [n_tok_pc, OUT_F], fp32, isOutput=True)

    with tile.TileContext(nc) as tc, ExitStack() as ctx:
        consts = ctx.enter_context(tc.tile_pool(name="consts", bufs=1))
        xpool = ctx.enter_context(tc.tile_pool(name="xpool", bufs=1))
        wepool = ctx.enter_context(tc.tile_pool(name="wepool", bufs=2))
        opool = ctx.enter_context(tc.tile_pool(name="opool", bufs=1))
        gpool = ctx.enter_context(tc.tile_pool(name="gpool", bufs=1))
        small = ctx.enter_context(tc.tile_pool(name="small", bufs=4))
        # 8 PSUM banks: 5 x yh ([128,512] f32 half-accumulators — the deep
        # rotation pushes the block-start WAR ~2.5 blocks back so it never
        # reaches the PE) + 2 x pb (bias halves, double-buffered) + 1 x lgt.
        psum_yh = ctx.enter_context(tc.tile_pool(name="psum_yh", bufs=5, space="PSUM"))
        psum_pb = ctx.enter_context(tc.tile_pool(name="psum_pb", bufs=2, space="PSUM"))
        psum_g = ctx.enter_context(tc.tile_pool(name="psum_g", bufs=1, space="PSUM"))

        # ---- HAM warmup: start PE activity as early as possible ----
        # gpsimd memset is available right after the preamble barrier
        # (vector is still busy with its register loads then), so warm
        # matmuls can begin ~1us earlier. N=128 keeps each warm matmul
        # cheap so real matmuls slot in as soon as their data lands.
        warm_sb = consts.tile([P, P], bf16)
        nc.gpsimd.memset(warm_sb, 0.25)

        def warm_fill():
            # dep-free N=128 matmul: fills DMA-chase idle so the HAM clock
            # gate never sees an idle window. Writes a fresh yh slot; its
            # only accessor is the matmul itself, so the slot frees at once.
            wps = psum_yh.tile([P, P], fp32, tag="yh")
            nc.tensor.matmul(wps, lhsT=warm_sb, rhs=warm_sb, start=True, stop=True)

        for _ in range(32):
            warm_fill()

        # ---- input DMAs ----
        # small gate constants ride gpsimd's (software) DGE — fast for small
        # transfers and keeps the sync/scalar issue streams free for the bulk
        wg_sb = consts.tile([P, K_CH, E], bf16)
        nc.gpsimd.dma_start(out=wg_sb, in_=Wg[:, :, :])
        bgc_sb = consts.tile([E, 1], fp32)
        nc.gpsimd.dma_start(out=bgc_sb, in_=bgc[:, :])
        be_sb = consts.tile([E, OUT_F], bf16)
        nc.gpsimd.dma_start(out=be_sb, in_=be[:, :])

        def fetch_we_chunk(e, c, eng=None):
            # later-expert prefetches ride SYNC: their dma_starts embed long
            # wepool-WAR waits (the slot frees only when expert e-2 finishes
            # reading it), and on scalar those waits would block the ACT
            # FIFO ahead of the per-block bias copies for microseconds.
            wc = wepool.tile([P, OUT_F], bf16, tag=f"we{c}")
            (eng or nc.sync).dma_start(
                out=wc, in_=We[e, c * P : (c + 1) * P, :]
            )
            return wc

        def fetch_we(e):
            return [fetch_we_chunk(e, c) for c in range(K_CH)]

        # The start of the kernel is HBM-bound: block 0 needs the xA
        # half-chunks AND all of We[0] (3MB critical). xA rides sync alone
        # (so the gate chain gets chunks at a fast, even pace); We[0] is
        # split 6 on scalar + the last 2 on sync behind xA, so both queues
        # carry only critical bytes until block 0's data is in. xB (needed
        # from t=T/2) queues on scalar right after its critical share, and
        # We[1] (needed at the e=1 sweep) on sync.
        nh = n_tok_pc // 2
        T_half = T // 2
        xA_sb, xB_sb = [None] * K_CH, [None] * K_CH
        we_sb = {0: [None] * K_CH}
        for c in range(K_CH):
            xa = xpool.tile([P, nh], bf16, tag=f"xa{c}")
            nc.sync.dma_start(out=xa, in_=xT[c * P : (c + 1) * P, :nh])
            xA_sb[c] = xa
            if c < 6:
                we_sb[0][c] = fetch_we_chunk(0, c, eng=nc.scalar)
        for c in range(6, K_CH):
            we_sb[0][c] = fetch_we_chunk(0, c, eng=nc.sync)
        # scalar carries NOTHING after its 6 We[0] chunks: any dma_start on
        # it would sit in the ACT FIFO ahead of the per-block bias copies.
        # xB rides sync behind We[1]; it lands ~26us, needed at t=T/2 (~28).
        we_sb[1] = [fetch_we_chunk(1, c, eng=nc.sync) for c in range(K_CH)]
        for c in range(K_CH):
            xb = xpool.tile([P, nh], bf16, tag=f"xb{c}")
            nc.sync.dma_start(out=xb, in_=xT[c * P : (c + 1) * P, nh:])
            xB_sb[c] = xb

        def xslice(c, t):
            if t < T_half:
                return xA_sb[c][:, t * P : (t + 1) * P]
            return xB_sb[c][:, (t - T_half) * P : (t - T_half + 1) * P]

        g_sb = gpool.tile([P, T, E], fp32)
        gTexp_sb = gpool.tile([E, T, P], bf16)
        # transposed exp'd gate staging in f32: 32 partitions so the DVE
        # 32x32 block transpose can address it; partitions 8..31 are zero.
        gTexp32_sb = gpool.tile([32, T, P], fp32)
        nc.gpsimd.memset(gTexp32_sb, 0.0)
        r_sb = gpool.tile([P, T], fp32)
        # out staging split into 4 tiles (t mod 4): Tile tracks deps per
        # tile, so a single out tile would falsely serialize the ACT bias
        # copies against the DVE combines of *other* token tiles.
        out4 = []
        for i in range(4):
            o_i = opool.tile([P, T // 4, OUT_F], fp32, tag=f"out{i}")
            out4.append(o_i)

        def oview(t, s=slice(None)):
            return out4[t % 4][:, t // 4, s]

        def main_mms(e, t, warm=0):
            # h-outer: each 512-wide half accumulates in its own psum tile,
            # so the h0 half finishes (and its combine starts) mid-block.
            phs = []
            for h in range(2):
                ph = psum_yh.tile([P, 512], fp32, tag="yh")
                hs = slice(h * 512, (h + 1) * 512)
                for c in range(K_CH):
                    if h == 0:
                        # fills go BEFORE the matmul: the PE queue is FIFO,
                        # so a fill behind a DMA-stalled matmul can't run
                        for _ in range(warm):
                            warm_fill()
                    nc.tensor.matmul(
                        ph,
                        lhsT=xslice(c, t),
                        rhs=we_sb[e][c][:, hs],
                        start=(c == 0),
                        stop=(c == K_CH - 1),
                    )
                phs.append(ph)
            return phs

        lgt_live = {}

        def gate_mms_batch(b):
            # transposed gate logits for a whole token-half at once:
            # lhsT = Wg chunk (8-col weight load), rhs = xA/xB (N=nh moving)
            lgt = psum_g.tile([E, nh], fp32, tag="g8")
            half = xA_sb if b == 0 else xB_sb
            for c in range(K_CH):
                if b == 0:
                    # dep-free fillers BEFORE the (DMA-chasing) matmul: the
                    # PE queue is FIFO, so fills behind a stalled matmul
                    # can't keep the HAM clock gate warm
                    warm_fill()
                    warm_fill()
                    warm_fill()
                nc.tensor.matmul(
                    lgt,
                    lhsT=wg_sb[:, c, :],
                    rhs=half[c][:, :],
                    start=(c == 0),
                    stop=(c == K_CH - 1),
                )
            # += bg (per-partition scalar in transposed space)
            nc.vector.tensor_scalar_add(lgt, lgt, bgc_sb[:, :])
            lgt_live[b] = lgt

        def gate_exps(b):
            # unnormalized transposed exp (logits are O(+-3) so exp without
            # max-subtraction is safe in f32): bf16 copy feeds the bias
            # matmul, f32 copy feeds the per-tile DVE transpose + softmax.
            # Emitted a block after the batch matmuls so the ACT FIFO never
            # stalls on them ahead of the per-block bias copies.
            lgt = lgt_live.pop(b)
            bslc = slice(b * T_half, (b + 1) * T_half)
            nc.scalar.activation(
                out=gTexp_sb[:, bslc, :],
                in_=lgt,
                func=mybir.ActivationFunctionType.Exp,
            )
            nc.scalar.activation(
                out=gTexp32_sb[:E, bslc, :],
                in_=lgt,
                func=mybir.ActivationFunctionType.Exp,
            )

        def softmax(t):
            # DVE 32x32-block transpose of the exp'd gate back to [tok, e]
            # (4 blocks; input partitions 8..31 are zero so output columns
            # 8..31 are zero and unused), then an all-DVE normalize chain —
            # no scalar-engine hop on this path.
            e32 = small.tile([P, 32], fp32, tag="z32")
            for j in range(4):
                nc.vector.transpose(
                    out=e32[32 * j : 32 * (j + 1), 0:32],
                    in_=gTexp32_sb[:, t, 32 * j : 32 * (j + 1)],
                )
            ssum = small.tile([P, 1], fp32, tag="ssum")
            nc.vector.tensor_reduce(
                out=ssum, in_=e32[:, 0:E], axis=mybir.AxisListType.X,
                op=mybir.AluOpType.add,
            )
            nc.vector.reciprocal(out=r_sb[:, t : t + 1], in_=ssum)
            nc.vector.tensor_scalar_mul(g_sb[:, t, :], e32[:, 0:E], r_sb[:, t : t + 1])

        def bias_init(t):
            # unnormalized bias term exp(z) @ be, folded in normalized as
            # the INITIALIZER of out[t]: out[t] = Copy(pb * r) on the scalar
            # engine (per-partition scale); takes the bias fold off the
            # near-saturated DVE.
            for h in range(2):
                hs = slice(h * 512, (h + 1) * 512)
                pb = psum_pb.tile([P, 512], fp32, tag="pb")
                nc.tensor.matmul(
                    pb, lhsT=gTexp_sb[:, t, :], rhs=be_sb[:, hs],
                    start=True, stop=True,
                )
                nc.scalar.activation(
                    out=oview(t, hs),
                    in_=pb[:, :],
                    func=mybir.ActivationFunctionType.Copy,
                    scale=r_sb[:, t : t + 1],
                )

        def combine(e, t, phs):
            # out[t] = y(e) * g[:, t, e] + out[t]   (fused on DVE, per half;
            # for e=0 the bias term is already in out[t])
            for h, ph in enumerate(phs):
                hs = slice(h * 512, (h + 1) * 512)
                nc.vector.scalar_tensor_tensor(
                    out=oview(t, hs),
                    in0=ph[:, :],
                    scalar=g_sb[:, t, e : e + 1],
                    in1=oview(t, hs),
                    op0=mybir.AluOpType.mult,
                    op1=mybir.AluOpType.add,
                )

        combine0 = lambda t, phs: combine(0, t, phs)

        # ---- phase A: e=0 pipelined with the gate computation ----
        # block t emission order matters: the softmax chain must NOT queue
        # behind combine0 on DVE (combine0 depends on the ACT bias copy,
        # which depends on the previous recip — ordering softmax first keeps
        # the cross-engine chain's latency off the DVE FIFO head). The bias
        # matmuls+copies lead the block so their psum WAR resolves early:
        #   bias(t-1) | main(0,t) | gate mms | softmax(t) | combine0(t-1)
        py_live = {}
        gate_mms_batch(0)
        gate_exps(0)
        for t in range(T):
            if t >= 1:
                bias_init(t - 1)
            py_live[t] = main_mms(0, t, warm=(2 if t == 0 else 1 if t == 1 else 0))
            if t == max(0, T_half - 2):
                gate_mms_batch(1)
                gate_exps(1)
            softmax(t)
            if t >= 1:
                combine0(t - 1, py_live.pop(t - 1))

        # bridge: keep PE fed while softmax(T-1) resolves
        bias_init(T - 1)
        py_b = main_mms(1, 0)

        combine0(T - 1, py_live.pop(T - 1))
        combine(1, 0, py_b)

        # ---- phase B: experts 1..7 ----
        for e in range(1, E):
            if e + 1 < E:
                we_sb[e + 1] = fetch_we(e + 1)
            for t in range(1 if e == 1 else 0, T):
                if e == E - 1 and t == T - 1:
                    # final tile: 4 quarter chains (N=256) so each quarter's
                    # combine + write-back overlaps the next quarter's
                    # matmuls; the 5-deep yh rotation keeps every WAR off
                    # the PE.
                    for q in range(4):
                        qs = slice(q * 256, (q + 1) * 256)
                        pq = psum_yh.tile([P, 256], fp32, tag="yh")
                        for c in range(K_CH):
                            nc.tensor.matmul(
                                pq,
                                lhsT=xslice(c, t),
                                rhs=we_sb[e][c][:, qs],
                                start=(c == 0),
                                stop=(c == K_CH - 1),
                            )
                        nc.vector.scalar_tensor_tensor(
                            out=oview(t, qs),
                            in0=pq[:, :],
                            scalar=g_sb[:, t, e : e + 1],
                            in1=oview(t, qs),
                            op0=mybir.AluOpType.mult,
                            op1=mybir.AluOpType.add,
                        )
                        eng = nc.sync if q % 2 == 0 else nc.scalar
                        eng.dma_start(
                            out=out[t * P : (t + 1) * P, qs],
                            in_=oview(t, qs),
                        )
                else:
                    py = main_mms(e, t)
                    combine(e, t, py)
                    if e == E - 1:
                        # write back this tile right after its final combine
                        nc.sync.dma_start(
                            out=out[t * P : (t + 1) * P, :], in_=oview(t)
                        )
            del we_sb[e - 1]

    nc.compile()
    return nc


_NC_CACHE: dict = {}


def _get_nc(n_tok_pc: int):
    if n_tok_pc not in _NC_CACHE:
        _NC_CACHE[n_tok_pc] = build_nc(n_tok_pc)
    return _NC_CACHE[n_tok_pc]


def make_in_maps(x, We, be, Wg, bg):
    """Host-side sharding: token-shard + transpose x, bf16-cast everything."""
    bf16 = ml_dtypes.bfloat16
    x = np.asarray(x)
    n_tok_pc = x.shape[0] // N_CORES
    We_bf = np.asarray(We).astype(bf16)
    be_bf = np.asarray(be).astype(bf16)
    K_CH = IN_F // P
    # [1024, 8] -> [p, chunk, e]
    Wg_bf = (
        np.asarray(Wg).astype(bf16).reshape(K_CH, P, E).transpose(1, 0, 2).copy()
    )
    bg_col = np.asarray(bg).astype(np.float32).reshape(E, 1)
    xbf = x.astype(bf16)
    in_maps = []
    for cid in range(N_CORES):
        xs = xbf[cid * n_tok_pc : (cid + 1) * n_tok_pc]
        in_maps.append(
            {
                "xT": np.ascontiguousarray(xs.T),
                "We": We_bf,
                "be": be_bf,
                "Wg": Wg_bf,
                "bgc": bg_col,
            }
        )
    return in_maps, n_tok_pc


def run(x, We, be, Wg, bg, trace=False, **trace_kwargs):
    in_maps, n_tok_pc = make_in_maps(x, We, be, Wg, bg)
    nc = _get_nc(n_tok_pc)
    res = run_bass_kernel_spmd(
        nc, in_maps, core_ids=list(range(N_CORES)), trace=trace, **trace_kwargs
    )
    outs = [res.results[i]["out"] for i in range(N_CORES)]
    return np.concatenate(outs, axis=0), res


def kernel(x, We, be, Wg, bg):
    out, _ = run(x, We, be, Wg, bg, trace=False)
    return out
